# revision 3
# baseline (speedup 1.0000x reference)
"""Trainium2 Bass kernel for banded (episodic-memory) attention.

Module computation (B=4, S=4096, D=256, d2=512, band width 64):
    x = states.reshape(B, S, 512)
    q = x @ Wq.T ; k = x @ Wk.T
    scores = q @ k.T / sqrt(512), masked to j in [i-64, i-1]
    w = softmax(scores)  (fully-masked row 0 -> 0)
    retrieved = w @ x
    returns (retrieved.reshape(B,S,256,2), w)

Device strategy (8 cores = 4 batches x 2 sequence halves):
    scores[i,j] = x_i^T (Wq^T Wk) x_j / sqrt(512) = z_i . x_j with
    z = x @ (Wk^T Wq)^T  -- one fused projection instead of two, and the
    key side of the score matmul is x^T directly.  Per 128-query block the
    live key window is 256 wide (band is 64), so scores are a [128, 256]
    tile with a static additive mask; exp (no max-subtract needed: |s| is
    a few units at most) with fused row-sum; retrieval contracts the 256
    window against the values.  Unnormalized band + row sums go back to
    the host, which normalizes and scatters into the dense [S, S] output
    (all entries outside the band are exactly 0).  All matmuls run in
    float32r (TF32-like, ~1.5e-4 rel err, full PE rate at N>=256).
"""

import sys

if "/opt/trn_rl_repo" not in sys.path:
    sys.path.insert(0, "/opt/trn_rl_repo")

import numpy as np

B, S, D2 = 4, 4096, 512
BAND = 64
SH = S // 2          # 2048 rows per core
CTXP = SH + 2 * BAND  # 2176 padded context rows (= 17 * 128)
NBLK = SH // 128      # 16 query blocks per core
WIN = 256             # key window per query block
NEG = -1e30

_COMPILED = None


def _build_program():
    import concourse.tile as tile
    from concourse import bacc, mybir
    from concourse.masks import make_identity

    F32 = mybir.dt.float32
    F32R = mybir.dt.float32r
    Exp = mybir.ActivationFunctionType.Exp
    Copy = mybir.ActivationFunctionType.Copy

    nc = bacc.Bacc("TRN2", target_bir_lowering=False, debug=False)

    xT_d = nc.dram_tensor("xT", [D2, CTXP], F32R, kind="ExternalInput").ap()
    xv_d = nc.dram_tensor("xv", [CTXP, D2], F32R, kind="ExternalInput").ap()
    mT_d = nc.dram_tensor("mT", [D2, D2], F32R, kind="ExternalInput").ap()
    m0_d = nc.dram_tensor("m0", [128, WIN], F32, kind="ExternalInput").ap()
    ms_d = nc.dram_tensor("ms", [128, WIN], F32, kind="ExternalInput").ap()
    wb_d = nc.dram_tensor("wb", [SH, 192], F32, kind="ExternalOutput").ap()
    rs_d = nc.dram_tensor("rs", [128, NBLK], F32, kind="ExternalOutput").ap()
    ro_d = nc.dram_tensor("ro", [SH, D2], F32, kind="ExternalOutput").ap()

    with tile.TileContext(nc) as tc:
        with (
            tc.tile_pool(name="const", bufs=1) as cp,
            tc.tile_pool(name="spool", bufs=3) as s_pool,
            tc.tile_pool(name="wpool", bufs=3) as w_pool,
            tc.tile_pool(name="wtpool", bufs=3) as wt_pool,
            tc.tile_pool(name="ropool", bufs=3) as ro_pool,
            tc.tile_pool(name="stats", bufs=6) as st_pool,
            tc.tile_pool(name="pa", bufs=2, space="PSUM") as pa,
            tc.tile_pool(name="ps", bufs=2, space="PSUM") as ps,
            tc.tile_pool(name="pt", bufs=2, space="PSUM") as pt,
            tc.tile_pool(name="pr", bufs=2, space="PSUM") as pr,
        ):
            # ---- constant / input loads ----
            xT_sb = cp.tile([128, 4, CTXP], F32R)
            for dc in range(4):
                nc.sync.dma_start(xT_sb[:, dc, :], xT_d[dc * 128 : (dc + 1) * 128, :])
            xv_sb = cp.tile([128, CTXP // 128, D2], F32R)
            for t in range(CTXP // 128):
                nc.sync.dma_start(xv_sb[:, t, :], xv_d[t * 128 : (t + 1) * 128, :])
            mT_sb = cp.tile([128, 4, D2], F32R)
            for dc in range(4):
                nc.sync.dma_start(mT_sb[:, dc, :], mT_d[dc * 128 : (dc + 1) * 128, :])
            m0_sb = cp.tile([128, WIN], F32)
            nc.sync.dma_start(m0_sb[:], m0_d)
            ms_sb = cp.tile([128, WIN], F32)
            nc.sync.dma_start(ms_sb[:], ms_d)
            ident = cp.tile([128, 128], F32)
            make_identity(nc, ident[:])
            zt_sb = cp.tile([128, 4, SH], F32R)
            rs_sb = cp.tile([128, NBLK], F32)

            # ---- phase A: zT = (Wk^T Wq / sqrt(d2)) @ xT  (query cols) ----
            for st in range(4):
                for ec in range(4):
                    pz = pa.tile([128, 512], F32, tag="pz")
                    for dc in range(4):
                        nc.tensor.matmul(
                            pz[:],
                            mT_sb[:, dc, ec * 128 : (ec + 1) * 128],
                            xT_sb[:, dc, BAND + st * 512 : BAND + (st + 1) * 512],
                            start=(dc == 0),
                            stop=(dc == 3),
                        )
                    dst = zt_sb[:, ec, st * 512 : (st + 1) * 512]
                    if (st * 4 + ec) % 2 == 0:
                        nc.scalar.copy(dst, pz[:])
                    else:
                        nc.vector.tensor_copy(dst, pz[:])

            # ---- phase B: per 128-query block ----
            for p in range(NBLK):
                sps = ps.tile([128, WIN], F32, tag="sps")
                for cc in range(4):
                    nc.tensor.matmul(
                        sps[:],
                        zt_sb[:, cc, p * 128 : (p + 1) * 128],
                        xT_sb[:, cc, p * 128 : p * 128 + WIN],
                        start=(cc == 0),
                        stop=(cc == 3),
                    )
                mask = m0_sb if p == 0 else ms_sb
                s_sb = s_pool.tile([128, WIN], F32, tag="s")
                nc.vector.tensor_add(s_sb[:], sps[:], mask[:])
                w_sb = w_pool.tile([128, WIN], F32, tag="w")
                nc.scalar.activation(
                    w_sb[:], s_sb[:], Exp, accum_out=rs_sb[:, p : p + 1]
                )
                nc.sync.dma_start(wb_d[p * 128 : (p + 1) * 128, :], w_sb[:, 0:192])

                radd = st_pool.tile([128, 1], F32, tag="radd")
                nc.vector.tensor_scalar_add(radd[:], rs_sb[:, p : p + 1], 1e-30)
                rinv = st_pool.tile([128, 1], F32, tag="rinv")
                nc.vector.reciprocal(rinv[:], radd[:])

                tps = pt.tile([128, 2, 128], F32, tag="tps")
                nc.tensor.transpose(tps[:, 0, :], w_sb[:, 0:128], ident[:])
                nc.tensor.transpose(tps[:, 1, :], w_sb[:, 128:256], ident[:])
                wt_sb = wt_pool.tile([128, 2, 128], F32R, tag="wt")
                nc.vector.tensor_copy(wt_sb[:], tps[:])

                rps = pr.tile([128, D2], F32, tag="rps")
                nc.tensor.matmul(
                    rps[:], wt_sb[:, 0, :], xv_sb[:, p, :], start=True, stop=False
                )
                nc.tensor.matmul(
                    rps[:], wt_sb[:, 1, :], xv_sb[:, p + 1, :], start=False, stop=True
                )
                ro_sb = ro_pool.tile([128, D2], F32, tag="ro")
                nc.scalar.activation(ro_sb[:], rps[:], Copy, scale=rinv[:])
                nc.sync.dma_start(ro_d[p * 128 : (p + 1) * 128, :], ro_sb[:])

            nc.sync.dma_start(rs_d, rs_sb[:])

    nc.compile()
    return nc


def _get_program():
    global _COMPILED
    if _COMPILED is None:
        _COMPILED = _build_program()
    return _COMPILED


def _make_masks():
    qi = np.arange(128)[:, None]
    kj = np.arange(WIN)[None, :]
    band = (kj >= qi) & (kj <= qi + BAND - 1)
    mask_s = np.where(band, 0.0, NEG).astype(np.float32)
    mask_0 = np.where(band & (kj >= BAND), 0.0, NEG).astype(np.float32)
    return mask_0, mask_s


def kernel(states: np.ndarray, Wq: np.ndarray, Wk: np.ndarray):
    from concourse.bass_utils import run_bass_kernel_spmd

    x = np.ascontiguousarray(states.reshape(B, S, D2), dtype=np.float32)
    scale = np.float64(D2) ** -0.5
    mT = (
        (Wq.T.astype(np.float64) @ Wk.astype(np.float64)) * scale
    ).astype(np.float32)
    mask_0, mask_s = _make_masks()

    in_maps = []
    for c in range(8):
        b, h = c // 2, c % 2
        s0 = h * SH
        xpad = np.zeros((S + 2 * BAND, D2), dtype=np.float32)
        xpad[BAND : BAND + S] = x[b]
        xv = np.ascontiguousarray(xpad[s0 : s0 + CTXP])
        xT = np.ascontiguousarray(xv.T)
        in_maps.append(
            {
                "xT": xT,
                "xv": xv,
                "mT": mT,
                "m0": mask_0 if h == 0 else mask_s,
                "ms": mask_s,
            }
        )

    nc = _get_program()
    res = run_bass_kernel_spmd(nc, in_maps, core_ids=list(range(8))).results

    retrieved = np.empty((B, S, D2), dtype=np.float32)
    w = np.zeros((B, S, S), dtype=np.float32)
    for c in range(8):
        b, h = c // 2, c % 2
        s0 = h * SH
        r = res[c]
        retrieved[b, s0 : s0 + SH] = r["ro"]
        rsums = r["rs"].T.reshape(SH, 1)  # [p,qi] -> row p*128+qi
        wn = r["wb"] / (rsums + 1e-30)
        for p in range(NBLK):
            g0 = s0 + p * 128
            c0 = g0 - BAND
            blk = wn[p * 128 : (p + 1) * 128]
            if c0 < 0:
                w[b, g0 : g0 + 128, 0 : c0 + 192] = blk[:, -c0:]
            else:
                w[b, g0 : g0 + 128, c0 : c0 + 192] = blk
    retrieved[:, 0, :] = 0.0
    w[:, 0, :] = 0.0
    return retrieved.reshape(B, S, D2 // 2, 2), w


# revision 6
# speedup vs baseline: 68.1356x; 68.1356x over previous
"""Trainium2 Bass kernel for banded (episodic-memory) attention.

Module computation (B=4, S=4096, D=256, d2=512, band width 64):
    x = states.reshape(B, S, 512)
    q = x @ Wq.T ; k = x @ Wk.T
    scores = q @ k.T / sqrt(512), masked to j in [i-64, i-1]
    w = softmax(scores)  (fully-masked row 0 -> 0)
    retrieved = w @ x
    returns (retrieved.reshape(B,S,256,2), w)

Device strategy (8 cores = 4 batches x 2 sequence halves):
    scores[i,j] = x_i^T (Wq^T Wk) x_j / sqrt(512) = z_i . x_j with
    z_i = (Wq^T Wk)^T x_i -- one fused projection instead of two, and the
    key side of the score matmul is x^T directly.  Per 128-query block the
    live key window is 256 wide (band is 64), so scores are a [128, 256]
    tile with a static additive mask; exp (no max-subtract needed: |s| is
    a few units at most) with fused row-sum; retrieval contracts the 256
    window against the values.  Unnormalized band + row sums go back to
    the host, which normalizes and scatters into the dense [S, S] output
    (all entries outside the band are exactly 0).  All matmuls run in
    float32r (TF32-like, ~1.5e-4 rel err, full PE rate at N>=256).
"""

import sys

if "/opt/trn_rl_repo" not in sys.path:
    sys.path.insert(0, "/opt/trn_rl_repo")

import numpy as np

B, S, D2 = 4, 4096, 512
BAND = 64
SH = S // 2          # 2048 rows per core
CTXP = SH + 2 * BAND  # 2176 padded context rows (= 17 * 128)
NBLK = SH // 128      # 16 query blocks per core
WIN = 256             # key window per query block
NEG = -1e30

_PROGRAMS = {}


def _build_program(reps: int = 1):
    # reps > 1 repeats the whole compute body (same inputs/outputs) so a
    # benchmark can difference wall times to isolate device exec time.
    import concourse.tile as tile
    from concourse import bacc, mybir
    from concourse.masks import make_identity

    F32 = mybir.dt.float32
    F32R = mybir.dt.float32r
    Exp = mybir.ActivationFunctionType.Exp
    Copy = mybir.ActivationFunctionType.Copy

    nc = bacc.Bacc("TRN2", target_bir_lowering=False, debug=False)

    xT_d = nc.dram_tensor("xT", [D2, CTXP], F32R, kind="ExternalInput").ap()
    xv_d = nc.dram_tensor("xv", [CTXP, D2], F32R, kind="ExternalInput").ap()
    mT_d = nc.dram_tensor("mT", [D2, D2], F32R, kind="ExternalInput").ap()
    m0_d = nc.dram_tensor("m0", [128, WIN], F32, kind="ExternalInput").ap()
    ms_d = nc.dram_tensor("ms", [128, WIN], F32, kind="ExternalInput").ap()
    wb_d = nc.dram_tensor("wb", [SH, 192], F32, kind="ExternalOutput").ap()
    rs_d = nc.dram_tensor("rs", [128, NBLK], F32, kind="ExternalOutput").ap()
    ro_d = nc.dram_tensor("ro", [SH, D2], F32, kind="ExternalOutput").ap()

    with tile.TileContext(nc) as tc:
        with (
            tc.tile_pool(name="const", bufs=1) as cp,
            tc.tile_pool(name="spool", bufs=3) as s_pool,
            tc.tile_pool(name="wpool", bufs=3) as w_pool,
            tc.tile_pool(name="wtpool", bufs=3) as wt_pool,
            tc.tile_pool(name="ropool", bufs=3) as ro_pool,
            tc.tile_pool(name="stats", bufs=6) as st_pool,
            tc.tile_pool(name="pa", bufs=2, space="PSUM") as pa,
            tc.tile_pool(name="ps", bufs=2, space="PSUM") as ps,
            tc.tile_pool(name="pt", bufs=2, space="PSUM") as pt,
            tc.tile_pool(name="pr", bufs=2, space="PSUM") as pr,
        ):
            # ---- constant / input loads ----
            xT_sb = cp.tile([128, 4, CTXP], F32R)
            for dc in range(4):
                nc.sync.dma_start(xT_sb[:, dc, :], xT_d[dc * 128 : (dc + 1) * 128, :])
            xv_sb = cp.tile([128, CTXP // 128, D2], F32R)
            for t in range(CTXP // 128):
                nc.sync.dma_start(xv_sb[:, t, :], xv_d[t * 128 : (t + 1) * 128, :])
            mT_sb = cp.tile([128, 4, D2], F32R)
            for dc in range(4):
                nc.sync.dma_start(mT_sb[:, dc, :], mT_d[dc * 128 : (dc + 1) * 128, :])
            m0_sb = cp.tile([128, WIN], F32)
            nc.sync.dma_start(m0_sb[:], m0_d)
            ms_sb = cp.tile([128, WIN], F32)
            nc.sync.dma_start(ms_sb[:], ms_d)
            ident = cp.tile([128, 128], F32)
            make_identity(nc, ident[:])
            zt_sb = cp.tile([128, 4, SH], F32R)
            rs_sb = cp.tile([128, NBLK], F32)

            for _rep in range(reps):
                # ---- phase A: zT = (Wq^T Wk / sqrt(d2))^T-proj of queries ----
                for st in range(4):
                    for ec in range(4):
                        pz = pa.tile([128, 512], F32, tag="pz", name="pz")
                        for dc in range(4):
                            nc.tensor.matmul(
                                pz[:],
                                mT_sb[:, dc, ec * 128 : (ec + 1) * 128],
                                xT_sb[:, dc, BAND + st * 512 : BAND + (st + 1) * 512],
                                start=(dc == 0),
                                stop=(dc == 3),
                            )
                        dst = zt_sb[:, ec, st * 512 : (st + 1) * 512]
                        if (st * 4 + ec) % 2 == 0:
                            nc.scalar.copy(dst, pz[:])
                        else:
                            nc.vector.tensor_copy(dst, pz[:])

                # ---- phase B: per 128-query block ----
                for p in range(NBLK):
                    sps = ps.tile([128, WIN], F32, tag="sps", name="sps")
                    for cc in range(4):
                        nc.tensor.matmul(
                            sps[:],
                            zt_sb[:, cc, p * 128 : (p + 1) * 128],
                            xT_sb[:, cc, p * 128 : p * 128 + WIN],
                            start=(cc == 0),
                            stop=(cc == 3),
                        )
                    mask = m0_sb if p == 0 else ms_sb
                    s_sb = s_pool.tile([128, WIN], F32, tag="s", name="s_sb")
                    nc.vector.tensor_add(s_sb[:], sps[:], mask[:])
                    w_sb = w_pool.tile([128, WIN], F32, tag="w", name="w_sb")
                    nc.scalar.activation(
                        w_sb[:], s_sb[:], Exp, accum_out=rs_sb[:, p : p + 1]
                    )
                    nc.sync.dma_start(wb_d[p * 128 : (p + 1) * 128, :], w_sb[:, 0:192])

                    radd = st_pool.tile([128, 1], F32, tag="radd", name="radd")
                    nc.vector.tensor_scalar_add(radd[:], rs_sb[:, p : p + 1], 1e-30)
                    rinv = st_pool.tile([128, 1], F32, tag="rinv", name="rinv")
                    nc.vector.reciprocal(rinv[:], radd[:])

                    tps = pt.tile([128, 2, 128], F32, tag="tps", name="tps")
                    nc.tensor.transpose(tps[:, 0, :], w_sb[:, 0:128], ident[:])
                    nc.tensor.transpose(tps[:, 1, :], w_sb[:, 128:256], ident[:])
                    wt_sb = wt_pool.tile([128, 2, 128], F32R, tag="wt", name="wt_sb")
                    nc.vector.tensor_copy(wt_sb[:], tps[:])

                    rps = pr.tile([128, D2], F32, tag="rps", name="rps")
                    nc.tensor.matmul(
                        rps[:], wt_sb[:, 0, :], xv_sb[:, p, :], start=True, stop=False
                    )
                    nc.tensor.matmul(
                        rps[:], wt_sb[:, 1, :], xv_sb[:, p + 1, :], start=False, stop=True
                    )
                    ro_sb = ro_pool.tile([128, D2], F32, tag="ro", name="ro_sb")
                    nc.scalar.activation(ro_sb[:], rps[:], Copy, scale=rinv[:])
                    nc.sync.dma_start(ro_d[p * 128 : (p + 1) * 128, :], ro_sb[:])

                nc.sync.dma_start(rs_d, rs_sb[:])

    nc.compile()
    return nc


def _get_program(reps: int = 1):
    if reps not in _PROGRAMS:
        _PROGRAMS[reps] = _build_program(reps)
    return _PROGRAMS[reps]


def _make_masks():
    qi = np.arange(128)[:, None]
    kj = np.arange(WIN)[None, :]
    band = (kj >= qi) & (kj <= qi + BAND - 1)
    mask_s = np.where(band, 0.0, NEG).astype(np.float32)
    mask_0 = np.where(band & (kj >= BAND), 0.0, NEG).astype(np.float32)
    return mask_0, mask_s


def _make_in_maps(states, Wq, Wk):
    x = np.ascontiguousarray(states.reshape(B, S, D2), dtype=np.float32)
    scale = np.float64(D2) ** -0.5
    mT = ((Wq.T.astype(np.float64) @ Wk.astype(np.float64)) * scale).astype(np.float32)
    mask_0, mask_s = _make_masks()

    in_maps = []
    for c in range(8):
        b, h = c // 2, c % 2
        s0 = h * SH
        xpad = np.zeros((S + 2 * BAND, D2), dtype=np.float32)
        xpad[BAND : BAND + S] = x[b]
        xv = np.ascontiguousarray(xpad[s0 : s0 + CTXP])
        xT = np.ascontiguousarray(xv.T)
        in_maps.append(
            {
                "xT": xT,
                "xv": xv,
                "mT": mT,
                "m0": mask_0 if h == 0 else mask_s,
                "ms": mask_s,
            }
        )
    return in_maps


def _assemble(res):
    retrieved = np.empty((B, S, D2), dtype=np.float32)
    w = np.zeros((B, S, S), dtype=np.float32)
    for c in range(8):
        b, h = c // 2, c % 2
        s0 = h * SH
        r = res[c]
        retrieved[b, s0 : s0 + SH] = r["ro"]
        rsums = r["rs"].T.reshape(SH, 1)  # [qi, p] -> row p*128+qi
        wn = r["wb"] / (rsums + 1e-30)
        for p in range(NBLK):
            g0 = s0 + p * 128
            c0 = g0 - BAND
            blk = wn[p * 128 : (p + 1) * 128]
            if c0 < 0:
                w[b, g0 : g0 + 128, 0 : c0 + 192] = blk[:, -c0:]
            else:
                w[b, g0 : g0 + 128, c0 : c0 + 192] = blk
    retrieved[:, 0, :] = 0.0
    w[:, 0, :] = 0.0
    return retrieved.reshape(B, S, D2 // 2, 2), w


def kernel(states: np.ndarray, Wq: np.ndarray, Wk: np.ndarray):
    from concourse.bass_utils import run_bass_kernel_spmd

    in_maps = _make_in_maps(states, Wq, Wk)
    nc = _get_program()
    res = run_bass_kernel_spmd(nc, in_maps, core_ids=list(range(8))).results
    return _assemble(res)


# revision 8
# speedup vs baseline: 4130.7371x; 60.6252x over previous
"""Trainium2 Bass kernel for banded (episodic-memory) attention.

Module computation (B=4, S=4096, D=256, d2=512, band width 64):
    x = states.reshape(B, S, 512)
    q = x @ Wq.T ; k = x @ Wk.T
    scores = q @ k.T / sqrt(512), masked to j in [i-64, i-1]
    w = softmax(scores)  (fully-masked row 0 -> 0)
    retrieved = w @ x
    returns (retrieved.reshape(B,S,256,2), w)

Device strategy (8 cores = 4 batches x 2 sequence halves):
    scores[i,j] = x_i^T (Wq^T Wk) x_j / sqrt(512) = z_i . x_j with
    z_i = (Wq^T Wk)^T x_i -- one fused projection instead of two, and the
    key side of the score matmul is x^T directly.  Per 128-query block the
    live key window is 256 wide (band is 64), so scores are a [128, 256]
    tile with a static additive mask; exp (no max-subtract needed: |s| is
    a few units at most) with fused row-sum; retrieval contracts the 256
    window against the values.  Unnormalized band + row sums go back to
    the host, which normalizes and scatters into the dense [S, S] output
    (all entries outside the band are exactly 0).  All matmuls run in
    float32r (TF32-like, ~1.5e-4 rel err, full PE rate at N>=256).
"""

import sys
from contextlib import ExitStack

if "/opt/trn_rl_repo" not in sys.path:
    sys.path.insert(0, "/opt/trn_rl_repo")

import numpy as np

B, S, D2 = 4, 4096, 512
BAND = 64
SH = S // 2          # 2048 rows per core
CTXP = SH + 2 * BAND  # 2176 padded context rows (= 17 * 128)
NBLK = SH // 128      # 16 query blocks per core
WIN = 256             # key window per query block
NEG = -1e30

_PROGRAMS = {}


def _build_program(reps: int = 1):
    # reps > 1 repeats the whole compute body (same inputs/outputs) so a
    # benchmark can difference wall times to isolate device exec time.
    import concourse.tile as tile
    from concourse import bacc, mybir
    from concourse.masks import make_identity

    F32 = mybir.dt.float32
    F32R = mybir.dt.float32r
    Exp = mybir.ActivationFunctionType.Exp
    Copy = mybir.ActivationFunctionType.Copy

    nc = bacc.Bacc("TRN2", target_bir_lowering=False, debug=False)

    xT_d = nc.dram_tensor("xT", [D2, CTXP], F32R, kind="ExternalInput").ap()
    xv_d = nc.dram_tensor("xv", [CTXP, D2], F32R, kind="ExternalInput").ap()
    mT_d = nc.dram_tensor("mT", [D2, D2], F32R, kind="ExternalInput").ap()
    m0_d = nc.dram_tensor("m0", [128, WIN], F32, kind="ExternalInput").ap()
    ms_d = nc.dram_tensor("ms", [128, WIN], F32, kind="ExternalInput").ap()
    wb_d = nc.dram_tensor("wb", [SH, 192], F32, kind="ExternalOutput").ap()
    rs_d = nc.dram_tensor("rs", [128, NBLK], F32, kind="ExternalOutput").ap()
    ro_d = nc.dram_tensor("ro", [SH, D2], F32, kind="ExternalOutput").ap()

    with tile.TileContext(nc) as tc:
        with (
            tc.tile_pool(name="const", bufs=1) as cp,
            tc.tile_pool(name="spool", bufs=3) as s_pool,
            tc.tile_pool(name="wpool", bufs=3) as w_pool,
            tc.tile_pool(name="wtpool", bufs=3) as wt_pool,
            tc.tile_pool(name="ropool", bufs=3) as ro_pool,
            tc.tile_pool(name="stats", bufs=6) as st_pool,
            tc.tile_pool(name="pa", bufs=2, space="PSUM") as pa,
            tc.tile_pool(name="ps", bufs=2, space="PSUM") as ps,
            tc.tile_pool(name="pt", bufs=2, space="PSUM") as pt,
            tc.tile_pool(name="pr", bufs=2, space="PSUM") as pr,
            ExitStack() as loop_ctx,
        ):
            ident = cp.tile([128, 128], F32)
            make_identity(nc, ident[:])
            xT_sb = cp.tile([128, 4, CTXP], F32R)
            xv_sb = cp.tile([128, CTXP // 128, D2], F32R)
            mT_sb = cp.tile([128, 4, D2], F32R)
            m0_sb = cp.tile([128, WIN], F32)
            ms_sb = cp.tile([128, WIN], F32)
            zt_sb = cp.tile([128, 4, SH], F32R)
            rs_sb = cp.tile([128, NBLK], F32)

            if reps > 1:
                loop_ctx.enter_context(tc.For_i(0, reps, 1))

            if True:
                # ---- input loads (inside the bench loop so per-iteration
                # time includes the real input DMA cost) ----
                for dc in range(4):
                    nc.sync.dma_start(
                        xT_sb[:, dc, :], xT_d[dc * 128 : (dc + 1) * 128, :]
                    )
                for t in range(CTXP // 128):
                    nc.sync.dma_start(xv_sb[:, t, :], xv_d[t * 128 : (t + 1) * 128, :])
                for dc in range(4):
                    nc.sync.dma_start(
                        mT_sb[:, dc, :], mT_d[dc * 128 : (dc + 1) * 128, :]
                    )
                nc.sync.dma_start(m0_sb[:], m0_d)
                nc.sync.dma_start(ms_sb[:], ms_d)

                # ---- phase A: zT = (Wq^T Wk / sqrt(d2))^T-proj of queries ----
                for st in range(4):
                    for ec in range(4):
                        pz = pa.tile([128, 512], F32, tag="pz", name="pz")
                        for dc in range(4):
                            nc.tensor.matmul(
                                pz[:],
                                mT_sb[:, dc, ec * 128 : (ec + 1) * 128],
                                xT_sb[:, dc, BAND + st * 512 : BAND + (st + 1) * 512],
                                start=(dc == 0),
                                stop=(dc == 3),
                            )
                        dst = zt_sb[:, ec, st * 512 : (st + 1) * 512]
                        if (st * 4 + ec) % 2 == 0:
                            nc.scalar.copy(dst, pz[:])
                        else:
                            nc.vector.tensor_copy(dst, pz[:])

                # ---- phase B: per 128-query block ----
                for p in range(NBLK):
                    sps = ps.tile([128, WIN], F32, tag="sps", name="sps")
                    for cc in range(4):
                        nc.tensor.matmul(
                            sps[:],
                            zt_sb[:, cc, p * 128 : (p + 1) * 128],
                            xT_sb[:, cc, p * 128 : p * 128 + WIN],
                            start=(cc == 0),
                            stop=(cc == 3),
                        )
                    mask = m0_sb if p == 0 else ms_sb
                    s_sb = s_pool.tile([128, WIN], F32, tag="s", name="s_sb")
                    nc.vector.tensor_add(s_sb[:], sps[:], mask[:])
                    w_sb = w_pool.tile([128, WIN], F32, tag="w", name="w_sb")
                    nc.scalar.activation(
                        w_sb[:], s_sb[:], Exp, accum_out=rs_sb[:, p : p + 1]
                    )
                    nc.sync.dma_start(wb_d[p * 128 : (p + 1) * 128, :], w_sb[:, 0:192])

                    radd = st_pool.tile([128, 1], F32, tag="radd", name="radd")
                    nc.vector.tensor_scalar_add(radd[:], rs_sb[:, p : p + 1], 1e-30)
                    rinv = st_pool.tile([128, 1], F32, tag="rinv", name="rinv")
                    nc.vector.reciprocal(rinv[:], radd[:])

                    tps = pt.tile([128, 2, 128], F32, tag="tps", name="tps")
                    nc.tensor.transpose(tps[:, 0, :], w_sb[:, 0:128], ident[:])
                    nc.tensor.transpose(tps[:, 1, :], w_sb[:, 128:256], ident[:])
                    wt_sb = wt_pool.tile([128, 2, 128], F32R, tag="wt", name="wt_sb")
                    nc.vector.tensor_copy(wt_sb[:], tps[:])

                    rps = pr.tile([128, D2], F32, tag="rps", name="rps")
                    nc.tensor.matmul(
                        rps[:], wt_sb[:, 0, :], xv_sb[:, p, :], start=True, stop=False
                    )
                    nc.tensor.matmul(
                        rps[:], wt_sb[:, 1, :], xv_sb[:, p + 1, :], start=False, stop=True
                    )
                    ro_sb = ro_pool.tile([128, D2], F32, tag="ro", name="ro_sb")
                    nc.scalar.activation(ro_sb[:], rps[:], Copy, scale=rinv[:])
                    nc.sync.dma_start(ro_d[p * 128 : (p + 1) * 128, :], ro_sb[:])

                nc.sync.dma_start(rs_d, rs_sb[:])

    nc.compile()
    return nc


def _get_program(reps: int = 1):
    if reps not in _PROGRAMS:
        _PROGRAMS[reps] = _build_program(reps)
    return _PROGRAMS[reps]


def _make_masks():
    qi = np.arange(128)[:, None]
    kj = np.arange(WIN)[None, :]
    band = (kj >= qi) & (kj <= qi + BAND - 1)
    mask_s = np.where(band, 0.0, NEG).astype(np.float32)
    mask_0 = np.where(band & (kj >= BAND), 0.0, NEG).astype(np.float32)
    return mask_0, mask_s


def _make_in_maps(states, Wq, Wk):
    x = np.ascontiguousarray(states.reshape(B, S, D2), dtype=np.float32)
    scale = np.float64(D2) ** -0.5
    mT = ((Wq.T.astype(np.float64) @ Wk.astype(np.float64)) * scale).astype(np.float32)
    mask_0, mask_s = _make_masks()

    in_maps = []
    for c in range(8):
        b, h = c // 2, c % 2
        s0 = h * SH
        xpad = np.zeros((S + 2 * BAND, D2), dtype=np.float32)
        xpad[BAND : BAND + S] = x[b]
        xv = np.ascontiguousarray(xpad[s0 : s0 + CTXP])
        xT = np.ascontiguousarray(xv.T)
        in_maps.append(
            {
                "xT": xT,
                "xv": xv,
                "mT": mT,
                "m0": mask_0 if h == 0 else mask_s,
                "ms": mask_s,
            }
        )
    return in_maps


def _assemble(res):
    retrieved = np.empty((B, S, D2), dtype=np.float32)
    w = np.zeros((B, S, S), dtype=np.float32)
    for c in range(8):
        b, h = c // 2, c % 2
        s0 = h * SH
        r = res[c]
        retrieved[b, s0 : s0 + SH] = r["ro"]
        rsums = r["rs"].T.reshape(SH, 1)  # [qi, p] -> row p*128+qi
        wn = r["wb"] / (rsums + 1e-30)
        for p in range(NBLK):
            g0 = s0 + p * 128
            c0 = g0 - BAND
            blk = wn[p * 128 : (p + 1) * 128]
            if c0 < 0:
                w[b, g0 : g0 + 128, 0 : c0 + 192] = blk[:, -c0:]
            else:
                w[b, g0 : g0 + 128, c0 : c0 + 192] = blk
    retrieved[:, 0, :] = 0.0
    w[:, 0, :] = 0.0
    return retrieved.reshape(B, S, D2 // 2, 2), w


def kernel(states: np.ndarray, Wq: np.ndarray, Wk: np.ndarray):
    from concourse.bass_utils import run_bass_kernel_spmd

    in_maps = _make_in_maps(states, Wq, Wk)
    nc = _get_program()
    res = run_bass_kernel_spmd(nc, in_maps, core_ids=list(range(8))).results
    return _assemble(res)


# revision 11
# speedup vs baseline: 8660.8294x; 2.0967x over previous
"""Trainium2 Bass kernel for banded (episodic-memory) attention.

Module computation (B=4, S=4096, D=256, d2=512, band width 64):
    x = states.reshape(B, S, 512)
    q = x @ Wq.T ; k = x @ Wk.T
    scores = q @ k.T / sqrt(512), masked to j in [i-64, i-1]
    w = softmax(scores)  (fully-masked row 0 -> 0)
    retrieved = w @ x
    returns (retrieved.reshape(B,S,256,2), w)

Device strategy (8 cores = 4 batches x 2 sequence halves):
    scores[i,j] = x_i^T (Wq^T Wk) x_j / sqrt(512) = z_i . x_j with
    z_i = (Wq^T Wk)^T x_i -- one fused projection instead of two, and the
    key side of the score matmul is x^T directly.  Per 128-query block the
    live key window is 256 wide (band is 64), so scores are a [128, 256]
    tile with a static additive mask; exp (no max-subtract needed: |s| is
    a few units at most) with fused row-sum; retrieval contracts the 256
    window against the values.  Unnormalized band + row sums go back to
    the host, which normalizes and scatters into the dense [S, S] output
    (all entries outside the band are exactly 0).  All matmuls run in
    float32r (TF32-like, ~1.5e-4 rel err, full PE rate at N>=256).
"""

import sys
from contextlib import ExitStack

if "/opt/trn_rl_repo" not in sys.path:
    sys.path.insert(0, "/opt/trn_rl_repo")

import numpy as np

B, S, D2 = 4, 4096, 512
BAND = 64
SH = S // 2          # 2048 rows per core
CTXP = SH + 2 * BAND  # 2176 padded context rows (= 17 * 128)
NBLK = SH // 128      # 16 query blocks per core
WIN = 256             # key window per query block
NEG = -1e30

_PROGRAMS = {}


def _build_program(reps: int = 1, probe: str | None = None):
    # reps > 1 repeats the whole compute body (same inputs/outputs) inside an
    # on-device For_i loop so a benchmark can difference wall times to
    # isolate per-iteration device time.
    # probe: None (full kernel) | "io" (DMAs only) | "compute" (no DMAs).
    do_io = probe != "compute"
    do_compute = probe != "io"
    import concourse.tile as tile
    from concourse import bacc, mybir
    from concourse.masks import make_identity

    F32 = mybir.dt.float32
    F32R = mybir.dt.float32r
    Exp = mybir.ActivationFunctionType.Exp
    Copy = mybir.ActivationFunctionType.Copy

    nc = bacc.Bacc("TRN2", target_bir_lowering=False, debug=False)

    xT_d = nc.dram_tensor("xT", [D2, CTXP], F32R, kind="ExternalInput").ap()
    xv_d = nc.dram_tensor("xv", [CTXP, D2], F32R, kind="ExternalInput").ap()
    mT_d = nc.dram_tensor("mT", [D2, D2], F32R, kind="ExternalInput").ap()
    m0_d = nc.dram_tensor("m0", [128, WIN], F32, kind="ExternalInput").ap()
    ms_d = nc.dram_tensor("ms", [128, WIN], F32, kind="ExternalInput").ap()
    wb_d = nc.dram_tensor("wb", [SH, 192], F32, kind="ExternalOutput").ap()
    rs_d = nc.dram_tensor("rs", [128, NBLK], F32, kind="ExternalOutput").ap()
    ro_d = nc.dram_tensor("ro", [SH, D2], F32, kind="ExternalOutput").ap()

    with tile.TileContext(nc) as tc:
        with (
            tc.tile_pool(name="const", bufs=1) as cp,
            tc.tile_pool(name="spool", bufs=3) as s_pool,
            tc.tile_pool(name="wpool", bufs=3) as w_pool,
            tc.tile_pool(name="wtpool", bufs=3) as wt_pool,
            tc.tile_pool(name="ropool", bufs=3) as ro_pool,
            tc.tile_pool(name="stats", bufs=6) as st_pool,
            tc.tile_pool(name="pa", bufs=2, space="PSUM") as pa,
            tc.tile_pool(name="ps", bufs=2, space="PSUM") as ps,
            tc.tile_pool(name="pt", bufs=2, space="PSUM") as pt,
            tc.tile_pool(name="pr", bufs=2, space="PSUM") as pr,
            ExitStack() as loop_ctx,
        ):
            ident = cp.tile([128, 128], F32)
            make_identity(nc, ident[:])
            xT_sb = cp.tile([128, 4, CTXP], F32R)
            xv_sb = cp.tile([128, CTXP // 128, D2], F32R)
            mT_sb = cp.tile([128, 4, D2], F32R)
            m0_sb = cp.tile([128, WIN], F32)
            ms_sb = cp.tile([128, WIN], F32)
            zt_sb = cp.tile([128, 4, SH], F32R)
            rs_sb = cp.tile([128, NBLK], F32)

            if reps > 1:
                loop_ctx.enter_context(tc.For_i(0, reps, 1, staggered_reset=True))

            # ---- input loads (inside the bench loop so per-iteration time
            # includes the real input DMA cost) ----
            if do_io:
                for dc in range(4):
                    nc.sync.dma_start(
                        xT_sb[:, dc, :], xT_d[dc * 128 : (dc + 1) * 128, :]
                    )
                for t in range(CTXP // 128):
                    nc.sync.dma_start(xv_sb[:, t, :], xv_d[t * 128 : (t + 1) * 128, :])
                for dc in range(4):
                    nc.sync.dma_start(
                        mT_sb[:, dc, :], mT_d[dc * 128 : (dc + 1) * 128, :]
                    )
                nc.sync.dma_start(m0_sb[:], m0_d)
                nc.sync.dma_start(ms_sb[:], ms_d)

            if do_compute:
                # ---- phase A: zT = (Wq^T Wk / sqrt(d2))^T-proj of queries ----
                for st in range(4):
                    for ec in range(4):
                        pz = pa.tile([128, 512], F32, tag="pz", name="pz")
                        for dc in range(4):
                            nc.tensor.matmul(
                                pz[:],
                                mT_sb[:, dc, ec * 128 : (ec + 1) * 128],
                                xT_sb[:, dc, BAND + st * 512 : BAND + (st + 1) * 512],
                                start=(dc == 0),
                                stop=(dc == 3),
                            )
                        dst = zt_sb[:, ec, st * 512 : (st + 1) * 512]
                        if (st * 4 + ec) % 2 == 0:
                            nc.scalar.copy(dst, pz[:])
                        else:
                            nc.vector.tensor_copy(dst, pz[:])

            # ---- phase B: per 128-query block ----
            for p in range(NBLK):
                if do_compute:
                    sps = ps.tile([128, WIN], F32, tag="sps", name="sps")
                    for cc in range(4):
                        nc.tensor.matmul(
                            sps[:],
                            zt_sb[:, cc, p * 128 : (p + 1) * 128],
                            xT_sb[:, cc, p * 128 : p * 128 + WIN],
                            start=(cc == 0),
                            stop=(cc == 3),
                        )
                    mask = m0_sb if p == 0 else ms_sb
                    s_sb = s_pool.tile([128, WIN], F32, tag="s", name="s_sb")
                    nc.vector.tensor_add(s_sb[:], sps[:], mask[:])
                    w_sb = w_pool.tile([128, WIN], F32, tag="w", name="w_sb")
                    nc.scalar.activation(
                        w_sb[:], s_sb[:], Exp, accum_out=rs_sb[:, p : p + 1]
                    )
                else:
                    w_sb = w_pool.tile([128, WIN], F32, tag="w", name="w_sb")
                if do_io:
                    nc.sync.dma_start(wb_d[p * 128 : (p + 1) * 128, :], w_sb[:, 0:192])

                if do_compute:
                    radd = st_pool.tile([128, 1], F32, tag="radd", name="radd")
                    nc.vector.tensor_scalar_add(radd[:], rs_sb[:, p : p + 1], 1e-30)
                    rinv = st_pool.tile([128, 1], F32, tag="rinv", name="rinv")
                    nc.vector.reciprocal(rinv[:], radd[:])

                    tps = pt.tile([128, 2, 128], F32, tag="tps", name="tps")
                    nc.tensor.transpose(tps[:, 0, :], w_sb[:, 0:128], ident[:])
                    nc.tensor.transpose(tps[:, 1, :], w_sb[:, 128:256], ident[:])
                    wt_sb = wt_pool.tile([128, 2, 128], F32R, tag="wt", name="wt_sb")
                    nc.vector.tensor_copy(wt_sb[:], tps[:])

                    rps = pr.tile([128, D2], F32, tag="rps", name="rps")
                    nc.tensor.matmul(
                        rps[:], wt_sb[:, 0, :], xv_sb[:, p, :], start=True, stop=False
                    )
                    nc.tensor.matmul(
                        rps[:],
                        wt_sb[:, 1, :],
                        xv_sb[:, p + 1, :],
                        start=False,
                        stop=True,
                    )
                    ro_sb = ro_pool.tile([128, D2], F32, tag="ro", name="ro_sb")
                    nc.scalar.activation(ro_sb[:], rps[:], Copy, scale=rinv[:])
                else:
                    ro_sb = ro_pool.tile([128, D2], F32, tag="ro", name="ro_sb")
                if do_io:
                    nc.sync.dma_start(ro_d[p * 128 : (p + 1) * 128, :], ro_sb[:])

            if do_io:
                nc.sync.dma_start(rs_d, rs_sb[:])

    nc.compile()
    return nc


def _get_program(reps: int = 1, probe: str | None = None):
    key = (reps, probe)
    if key not in _PROGRAMS:
        _PROGRAMS[key] = _build_program(reps, probe)
    return _PROGRAMS[key]


def _make_masks():
    qi = np.arange(128)[:, None]
    kj = np.arange(WIN)[None, :]
    band = (kj >= qi) & (kj <= qi + BAND - 1)
    mask_s = np.where(band, 0.0, NEG).astype(np.float32)
    mask_0 = np.where(band & (kj >= BAND), 0.0, NEG).astype(np.float32)
    return mask_0, mask_s


def _make_in_maps(states, Wq, Wk):
    x = np.ascontiguousarray(states.reshape(B, S, D2), dtype=np.float32)
    scale = np.float64(D2) ** -0.5
    mT = ((Wq.T.astype(np.float64) @ Wk.astype(np.float64)) * scale).astype(np.float32)
    mask_0, mask_s = _make_masks()

    in_maps = []
    for c in range(8):
        b, h = c // 2, c % 2
        s0 = h * SH
        xpad = np.zeros((S + 2 * BAND, D2), dtype=np.float32)
        xpad[BAND : BAND + S] = x[b]
        xv = np.ascontiguousarray(xpad[s0 : s0 + CTXP])
        xT = np.ascontiguousarray(xv.T)
        in_maps.append(
            {
                "xT": xT,
                "xv": xv,
                "mT": mT,
                "m0": mask_0 if h == 0 else mask_s,
                "ms": mask_s,
            }
        )
    return in_maps


def _assemble(res):
    retrieved = np.empty((B, S, D2), dtype=np.float32)
    w = np.zeros((B, S, S), dtype=np.float32)
    for c in range(8):
        b, h = c // 2, c % 2
        s0 = h * SH
        r = res[c]
        retrieved[b, s0 : s0 + SH] = r["ro"]
        rsums = r["rs"].T.reshape(SH, 1)  # [qi, p] -> row p*128+qi
        wn = r["wb"] / (rsums + 1e-30)
        for p in range(NBLK):
            g0 = s0 + p * 128
            c0 = g0 - BAND
            blk = wn[p * 128 : (p + 1) * 128]
            if c0 < 0:
                w[b, g0 : g0 + 128, 0 : c0 + 192] = blk[:, -c0:]
            else:
                w[b, g0 : g0 + 128, c0 : c0 + 192] = blk
    retrieved[:, 0, :] = 0.0
    w[:, 0, :] = 0.0
    return retrieved.reshape(B, S, D2 // 2, 2), w


def kernel(states: np.ndarray, Wq: np.ndarray, Wk: np.ndarray):
    from concourse.bass_utils import run_bass_kernel_spmd

    in_maps = _make_in_maps(states, Wq, Wk)
    nc = _get_program()
    res = run_bass_kernel_spmd(nc, in_maps, core_ids=list(range(8))).results
    return _assemble(res)


# revision 14
# speedup vs baseline: 38468.1615x; 4.4416x over previous
"""Trainium2 Bass kernel for banded (episodic-memory) attention.

Module computation (B=4, S=4096, D=256, d2=512, band width 64):
    x = states.reshape(B, S, 512)
    q = x @ Wq.T ; k = x @ Wk.T
    scores = q @ k.T / sqrt(512), masked to j in [i-64, i-1]
    w = softmax(scores)  (fully-masked row 0 -> 0)
    retrieved = w @ x
    returns (retrieved.reshape(B,S,256,2), w)

Device strategy (8 cores = 4 batches x 2 sequence halves):
    scores[i,j] = x_i^T (Wq^T Wk) x_j / sqrt(512) = z_i . x_j with
    z_i = (Wq^T Wk)^T x_i -- one fused projection instead of two, and the
    key side of the score matmul is x^T directly.  Per 128-query block the
    live key window is 256 wide (band is 64), so scores are a [128, 256]
    tile with a static additive mask; exp (no max-subtract needed: |s| is
    a few units at most) with fused row-sum; retrieval contracts the 256
    window against the values.  Unnormalized band + row sums go back to
    the host, which normalizes and scatters into the dense [S, S] output
    (all entries outside the band are exactly 0).  All matmuls run in
    float32r (TF32-like, ~1.5e-4 rel err, full PE rate at N>=256).
"""

import sys
from contextlib import ExitStack

if "/opt/trn_rl_repo" not in sys.path:
    sys.path.insert(0, "/opt/trn_rl_repo")

import numpy as np

B, S, D2 = 4, 4096, 512
BAND = 64
SH = S // 2          # 2048 rows per core
CTXP = SH + 2 * BAND  # 2176 padded context rows (= 17 * 128)
NBLK = SH // 128      # 16 query blocks per core
WIN = 256             # key window per query block
NEG = -1e30

_PROGRAMS = {}


def _build_program(reps: int = 1, probe: str | None = None):
    # reps > 1 repeats the whole compute body (same inputs/outputs) inside an
    # on-device For_i loop so a benchmark can difference wall times to
    # isolate per-iteration device time.
    # probe: None (full kernel) | "io" (DMAs only) | "compute" (no DMAs).
    do_io = probe != "compute"
    do_compute = probe != "io"
    import concourse.tile as tile
    from concourse import bacc, mybir
    from concourse.masks import make_identity

    F32 = mybir.dt.float32
    F32R = mybir.dt.float32r
    Exp = mybir.ActivationFunctionType.Exp
    Copy = mybir.ActivationFunctionType.Copy

    nc = bacc.Bacc("TRN2", target_bir_lowering=False, debug=False)

    xT_d = nc.dram_tensor("xT", [D2, CTXP], F32R, kind="ExternalInput").ap()
    xv_d = nc.dram_tensor("xv", [CTXP, D2], F32R, kind="ExternalInput").ap()
    mT_d = nc.dram_tensor("mT", [D2, D2], F32R, kind="ExternalInput").ap()
    m0_d = nc.dram_tensor("m0", [128, WIN], F32, kind="ExternalInput").ap()
    ms_d = nc.dram_tensor("ms", [128, WIN], F32, kind="ExternalInput").ap()
    wb_d = nc.dram_tensor("wb", [SH, 192], F32, kind="ExternalOutput").ap()
    rs_d = nc.dram_tensor("rs", [128, NBLK], F32, kind="ExternalOutput").ap()
    ro_d = nc.dram_tensor("ro", [SH, D2], F32, kind="ExternalOutput").ap()

    with tile.TileContext(nc) as tc:
        with (
            tc.tile_pool(name="const", bufs=1) as cp,
            tc.tile_pool(name="spool", bufs=3) as s_pool,
            tc.tile_pool(name="wpool", bufs=3) as w_pool,
            tc.tile_pool(name="wtpool", bufs=3) as wt_pool,
            tc.tile_pool(name="ropool", bufs=3) as ro_pool,
            tc.tile_pool(name="stats", bufs=6) as st_pool,
            tc.tile_pool(name="pa", bufs=2, space="PSUM") as pa,
            tc.tile_pool(name="ps", bufs=2, space="PSUM") as ps,
            tc.tile_pool(name="pt", bufs=2, space="PSUM") as pt,
            tc.tile_pool(name="pr", bufs=2, space="PSUM") as pr,
            ExitStack() as loop_ctx,
        ):
            ident = cp.tile([128, 128], F32)
            make_identity(nc, ident[:])
            xT_sb = cp.tile([128, 4, CTXP], F32R)
            xv_sb = cp.tile([128, CTXP // 128, D2], F32R)
            mT_sb = cp.tile([128, 4, D2], F32R)
            m0_sb = cp.tile([128, WIN], F32)
            ms_sb = cp.tile([128, WIN], F32)
            zt_sb = cp.tile([128, 4, SH], F32R)
            rs_sb = cp.tile([128, NBLK], F32)

            if not do_io:
                # compute-only probe: one-time zero-init so matmul inputs
                # have writers (outside the bench loop)
                for t in (xT_sb, xv_sb, mT_sb, m0_sb, ms_sb):
                    nc.gpsimd.memset(t[:], 0.0)

            if reps > 1:
                loop_ctx.enter_context(tc.For_i(0, reps, 1, staggered_reset=True))

            # ---- input loads (inside the bench loop so per-iteration time
            # includes the real input DMA cost) ----
            if do_io:
                for dc in range(4):
                    nc.sync.dma_start(
                        xT_sb[:, dc, :], xT_d[dc * 128 : (dc + 1) * 128, :]
                    )
                for t in range(CTXP // 128):
                    nc.sync.dma_start(xv_sb[:, t, :], xv_d[t * 128 : (t + 1) * 128, :])
                for dc in range(4):
                    nc.sync.dma_start(
                        mT_sb[:, dc, :], mT_d[dc * 128 : (dc + 1) * 128, :]
                    )
                nc.sync.dma_start(m0_sb[:], m0_d)
                nc.sync.dma_start(ms_sb[:], ms_d)

            if do_compute:
                # ---- phase A: zT = (Wq^T Wk / sqrt(d2))^T-proj of queries ----
                for st in range(4):
                    for ec in range(4):
                        pz = pa.tile([128, 512], F32, tag="pz", name="pz")
                        for dc in range(4):
                            nc.tensor.matmul(
                                pz[:],
                                mT_sb[:, dc, ec * 128 : (ec + 1) * 128],
                                xT_sb[:, dc, BAND + st * 512 : BAND + (st + 1) * 512],
                                start=(dc == 0),
                                stop=(dc == 3),
                            )
                        dst = zt_sb[:, ec, st * 512 : (st + 1) * 512]
                        if (st * 4 + ec) % 2 == 0:
                            nc.scalar.copy(dst, pz[:])
                        else:
                            nc.vector.tensor_copy(dst, pz[:])

            # ---- phase B: per 128-query block ----
            for p in range(NBLK):
                if do_compute:
                    sps = ps.tile([128, WIN], F32, tag="sps", name="sps")
                    for cc in range(4):
                        nc.tensor.matmul(
                            sps[:],
                            zt_sb[:, cc, p * 128 : (p + 1) * 128],
                            xT_sb[:, cc, p * 128 : p * 128 + WIN],
                            start=(cc == 0),
                            stop=(cc == 3),
                        )
                    mask = m0_sb if p == 0 else ms_sb
                    s_sb = s_pool.tile([128, WIN], F32, tag="s", name="s_sb")
                    nc.vector.tensor_add(s_sb[:], sps[:], mask[:])
                    w_sb = w_pool.tile([128, WIN], F32, tag="w", name="w_sb")
                    nc.scalar.activation(
                        w_sb[:], s_sb[:], Exp, accum_out=rs_sb[:, p : p + 1]
                    )
                if do_io:
                    wb_src = w_sb[:, 0:192] if do_compute else xv_sb[:, p, 0:192]
                    nc.sync.dma_start(wb_d[p * 128 : (p + 1) * 128, :], wb_src)

                if do_compute:
                    radd = st_pool.tile([128, 1], F32, tag="radd", name="radd")
                    nc.vector.tensor_scalar_add(radd[:], rs_sb[:, p : p + 1], 1e-30)
                    rinv = st_pool.tile([128, 1], F32, tag="rinv", name="rinv")
                    nc.vector.reciprocal(rinv[:], radd[:])

                    tps = pt.tile([128, 2, 128], F32, tag="tps", name="tps")
                    nc.tensor.transpose(tps[:, 0, :], w_sb[:, 0:128], ident[:])
                    nc.tensor.transpose(tps[:, 1, :], w_sb[:, 128:256], ident[:])
                    wt_sb = wt_pool.tile([128, 2, 128], F32R, tag="wt", name="wt_sb")
                    nc.vector.tensor_copy(wt_sb[:], tps[:])

                    rps = pr.tile([128, D2], F32, tag="rps", name="rps")
                    nc.tensor.matmul(
                        rps[:], wt_sb[:, 0, :], xv_sb[:, p, :], start=True, stop=False
                    )
                    nc.tensor.matmul(
                        rps[:],
                        wt_sb[:, 1, :],
                        xv_sb[:, p + 1, :],
                        start=False,
                        stop=True,
                    )
                    ro_sb = ro_pool.tile([128, D2], F32, tag="ro", name="ro_sb")
                    nc.scalar.activation(ro_sb[:], rps[:], Copy, scale=rinv[:])
                if do_io:
                    ro_src = ro_sb[:] if do_compute else xv_sb[:, p, :]
                    nc.sync.dma_start(ro_d[p * 128 : (p + 1) * 128, :], ro_src)

            if do_io:
                rs_src = rs_sb[:] if do_compute else xv_sb[:, 0, 0:NBLK]
                nc.sync.dma_start(rs_d, rs_src)

    nc.compile()
    return nc


def _get_program(reps: int = 1, probe: str | None = None):
    key = (reps, probe)
    if key not in _PROGRAMS:
        _PROGRAMS[key] = _build_program(reps, probe)
    return _PROGRAMS[key]


def _make_masks():
    qi = np.arange(128)[:, None]
    kj = np.arange(WIN)[None, :]
    band = (kj >= qi) & (kj <= qi + BAND - 1)
    mask_s = np.where(band, 0.0, NEG).astype(np.float32)
    mask_0 = np.where(band & (kj >= BAND), 0.0, NEG).astype(np.float32)
    return mask_0, mask_s


def _make_in_maps(states, Wq, Wk):
    x = np.ascontiguousarray(states.reshape(B, S, D2), dtype=np.float32)
    scale = np.float64(D2) ** -0.5
    mT = ((Wq.T.astype(np.float64) @ Wk.astype(np.float64)) * scale).astype(np.float32)
    mask_0, mask_s = _make_masks()

    in_maps = []
    for c in range(8):
        b, h = c // 2, c % 2
        s0 = h * SH
        xpad = np.zeros((S + 2 * BAND, D2), dtype=np.float32)
        xpad[BAND : BAND + S] = x[b]
        xv = np.ascontiguousarray(xpad[s0 : s0 + CTXP])
        xT = np.ascontiguousarray(xv.T)
        in_maps.append(
            {
                "xT": xT,
                "xv": xv,
                "mT": mT,
                "m0": mask_0 if h == 0 else mask_s,
                "ms": mask_s,
            }
        )
    return in_maps


def _assemble(res):
    retrieved = np.empty((B, S, D2), dtype=np.float32)
    w = np.zeros((B, S, S), dtype=np.float32)
    for c in range(8):
        b, h = c // 2, c % 2
        s0 = h * SH
        r = res[c]
        retrieved[b, s0 : s0 + SH] = r["ro"]
        rsums = r["rs"].T.reshape(SH, 1)  # [qi, p] -> row p*128+qi
        wn = r["wb"] / (rsums + 1e-30)
        for p in range(NBLK):
            g0 = s0 + p * 128
            c0 = g0 - BAND
            blk = wn[p * 128 : (p + 1) * 128]
            if c0 < 0:
                w[b, g0 : g0 + 128, 0 : c0 + 192] = blk[:, -c0:]
            else:
                w[b, g0 : g0 + 128, c0 : c0 + 192] = blk
    retrieved[:, 0, :] = 0.0
    w[:, 0, :] = 0.0
    return retrieved.reshape(B, S, D2 // 2, 2), w


def kernel(states: np.ndarray, Wq: np.ndarray, Wk: np.ndarray):
    from concourse.bass_utils import run_bass_kernel_spmd

    in_maps = _make_in_maps(states, Wq, Wk)
    nc = _get_program()
    res = run_bass_kernel_spmd(nc, in_maps, core_ids=list(range(8))).results
    return _assemble(res)


# revision 15
# speedup vs baseline: 61871.7678x; 1.6084x over previous
"""Trainium2 Bass kernel for banded (episodic-memory) attention.

Module computation (B=4, S=4096, D=256, d2=512, band width 64):
    x = states.reshape(B, S, 512)
    q = x @ Wq.T ; k = x @ Wk.T
    scores = q @ k.T / sqrt(512), masked to j in [i-64, i-1]
    w = softmax(scores)  (fully-masked row 0 -> 0)
    retrieved = w @ x
    returns (retrieved.reshape(B,S,256,2), w)

Device strategy (8 cores = 4 batches x 2 sequence halves):
    scores[i,j] = x_i^T (Wq^T Wk) x_j / sqrt(512) = z_i . x_j with
    z_i = (Wq^T Wk)^T x_i -- one fused projection instead of two, and the
    key side of the score matmul is x^T directly.  Per 128-query block the
    live key window is 256 wide (band is 64), so scores are a [128, 256]
    tile with a static additive mask; exp (no max-subtract needed: |s| is
    a few units at most) with fused row-sum; retrieval contracts the 256
    window against the values.  Unnormalized band + row sums go back to
    the host, which normalizes and scatters into the dense [S, S] output
    (all entries outside the band are exactly 0).  All matmuls run in
    float32r (TF32-like, ~1.5e-4 rel err, full PE rate at N>=256).
"""

import sys
from contextlib import ExitStack

if "/opt/trn_rl_repo" not in sys.path:
    sys.path.insert(0, "/opt/trn_rl_repo")

import numpy as np

B, S, D2 = 4, 4096, 512
BAND = 64
SH = S // 2          # 2048 rows per core
CTXP = SH + 2 * BAND  # 2176 padded context rows (= 17 * 128)
NBLK = SH // 128      # 16 query blocks per core
WIN = 256             # key window per query block
NEG = -1e30

_PROGRAMS = {}


def _build_program(reps: int = 1, probe: str | None = None):
    # reps > 1 repeats the whole compute body (same inputs/outputs) inside an
    # on-device For_i loop so a benchmark can difference wall times to
    # isolate per-iteration device time.
    # probe: None (full kernel) | "io" (DMAs only) | "compute" (no DMAs).
    do_io = probe != "compute"
    do_compute = probe != "io"
    import concourse.tile as tile
    from concourse import bacc, mybir
    from concourse.masks import make_identity

    F32 = mybir.dt.float32
    F32R = mybir.dt.float32r
    Exp = mybir.ActivationFunctionType.Exp
    Copy = mybir.ActivationFunctionType.Copy

    nc = bacc.Bacc("TRN2", target_bir_lowering=False, debug=False)

    xT_d = nc.dram_tensor("xT", [D2, CTXP], F32R, kind="ExternalInput").ap()
    xv_d = nc.dram_tensor("xv", [CTXP, D2], F32R, kind="ExternalInput").ap()
    mT_d = nc.dram_tensor("mT", [D2, D2], F32R, kind="ExternalInput").ap()
    m0_d = nc.dram_tensor("m0", [128, WIN], F32, kind="ExternalInput").ap()
    ms_d = nc.dram_tensor("ms", [128, WIN], F32, kind="ExternalInput").ap()
    wb_d = nc.dram_tensor("wb", [SH, 192], F32, kind="ExternalOutput").ap()
    rs_d = nc.dram_tensor("rs", [128, NBLK], F32, kind="ExternalOutput").ap()
    ro_d = nc.dram_tensor("ro", [SH, D2], F32, kind="ExternalOutput").ap()

    with tile.TileContext(nc) as tc:
        with (
            tc.tile_pool(name="const", bufs=1) as cp,
            tc.tile_pool(name="spool", bufs=3) as s_pool,
            tc.tile_pool(name="wpool", bufs=3) as w_pool,
            tc.tile_pool(name="wtpool", bufs=3) as wt_pool,
            tc.tile_pool(name="ropool", bufs=3) as ro_pool,
            tc.tile_pool(name="stats", bufs=6) as st_pool,
            tc.tile_pool(name="pa", bufs=2, space="PSUM") as pa,
            tc.tile_pool(name="ps", bufs=2, space="PSUM") as ps,
            tc.tile_pool(name="pt", bufs=2, space="PSUM") as pt,
            tc.tile_pool(name="pr", bufs=2, space="PSUM") as pr,
            ExitStack() as loop_ctx,
        ):
            ident = cp.tile([128, 128], F32)
            make_identity(nc, ident[:])
            xT_sb = cp.tile([128, 4, CTXP], F32R)
            xv_sb = cp.tile([128, CTXP // 128, D2], F32R)
            mT_sb = cp.tile([128, 4, D2], F32R)
            m0_sb = cp.tile([128, WIN], F32)
            ms_sb = cp.tile([128, WIN], F32)
            zt_sb = cp.tile([128, 4, SH], F32R)
            rs_sb = cp.tile([128, NBLK], F32)

            if not do_io:
                # compute-only probe: one-time zero-init so matmul inputs
                # have writers (outside the bench loop)
                for t in (xT_sb, xv_sb, mT_sb, m0_sb, ms_sb):
                    nc.gpsimd.memset(t[:], 0.0)

            if reps > 1:
                loop_ctx.enter_context(tc.For_i(0, reps, 1, staggered_reset=True))

            # ---- input loads (inside the bench loop so per-iteration time
            # includes the real input DMA cost) ----
            if do_io:
                for dc in range(4):
                    nc.sync.dma_start(
                        xT_sb[:, dc, :], xT_d[dc * 128 : (dc + 1) * 128, :]
                    )
                for t in range(CTXP // 128):
                    nc.sync.dma_start(xv_sb[:, t, :], xv_d[t * 128 : (t + 1) * 128, :])
                for dc in range(4):
                    nc.sync.dma_start(
                        mT_sb[:, dc, :], mT_d[dc * 128 : (dc + 1) * 128, :]
                    )
                nc.sync.dma_start(m0_sb[:], m0_d)
                nc.sync.dma_start(ms_sb[:], ms_d)

            if do_compute:
                # ---- phase A: zT = (Wq^T Wk / sqrt(d2))^T-proj of queries ----
                for st in range(4):
                    for ec in range(4):
                        pz = pa.tile([128, 512], F32, tag="pz", name="pz")
                        for dc in range(4):
                            nc.tensor.matmul(
                                pz[:],
                                mT_sb[:, dc, ec * 128 : (ec + 1) * 128],
                                xT_sb[:, dc, BAND + st * 512 : BAND + (st + 1) * 512],
                                start=(dc == 0),
                                stop=(dc == 3),
                            )
                        dst = zt_sb[:, ec, st * 512 : (st + 1) * 512]
                        if (st * 4 + ec) % 2 == 0:
                            nc.scalar.copy(dst, pz[:])
                        else:
                            nc.vector.tensor_copy(dst, pz[:])

            # ---- phase B: per 128-query block ----
            for p in range(NBLK):
                if do_compute:
                    sps = ps.tile([128, WIN], F32, tag="sps", name="sps")
                    for cc in range(4):
                        nc.tensor.matmul(
                            sps[:],
                            zt_sb[:, cc, p * 128 : (p + 1) * 128],
                            xT_sb[:, cc, p * 128 : p * 128 + WIN],
                            start=(cc == 0),
                            stop=(cc == 3),
                        )
                    mask = m0_sb if p == 0 else ms_sb
                    s_sb = s_pool.tile([128, WIN], F32, tag="s", name="s_sb")
                    nc.vector.tensor_add(s_sb[:], sps[:], mask[:])
                    w_sb = w_pool.tile([128, WIN], F32, tag="w", name="w_sb")
                    nc.scalar.activation(
                        w_sb[:], s_sb[:], Exp, accum_out=rs_sb[:, p : p + 1]
                    )
                if do_io:
                    wb_src = w_sb[:, 0:192] if do_compute else xv_sb[:, p, 0:192].bitcast(F32)
                    nc.sync.dma_start(wb_d[p * 128 : (p + 1) * 128, :], wb_src)

                if do_compute:
                    radd = st_pool.tile([128, 1], F32, tag="radd", name="radd")
                    nc.vector.tensor_scalar_add(radd[:], rs_sb[:, p : p + 1], 1e-30)
                    rinv = st_pool.tile([128, 1], F32, tag="rinv", name="rinv")
                    nc.vector.reciprocal(rinv[:], radd[:])

                    tps = pt.tile([128, 2, 128], F32, tag="tps", name="tps")
                    nc.tensor.transpose(tps[:, 0, :], w_sb[:, 0:128], ident[:])
                    nc.tensor.transpose(tps[:, 1, :], w_sb[:, 128:256], ident[:])
                    wt_sb = wt_pool.tile([128, 2, 128], F32R, tag="wt", name="wt_sb")
                    nc.vector.tensor_copy(wt_sb[:], tps[:])

                    rps = pr.tile([128, D2], F32, tag="rps", name="rps")
                    nc.tensor.matmul(
                        rps[:], wt_sb[:, 0, :], xv_sb[:, p, :], start=True, stop=False
                    )
                    nc.tensor.matmul(
                        rps[:],
                        wt_sb[:, 1, :],
                        xv_sb[:, p + 1, :],
                        start=False,
                        stop=True,
                    )
                    ro_sb = ro_pool.tile([128, D2], F32, tag="ro", name="ro_sb")
                    nc.scalar.activation(ro_sb[:], rps[:], Copy, scale=rinv[:])
                if do_io:
                    ro_src = ro_sb[:] if do_compute else xv_sb[:, p, :].bitcast(F32)
                    nc.sync.dma_start(ro_d[p * 128 : (p + 1) * 128, :], ro_src)

            if do_io:
                rs_src = rs_sb[:] if do_compute else xv_sb[:, 0, 0:NBLK].bitcast(F32)
                nc.sync.dma_start(rs_d, rs_src)

    nc.compile()
    return nc


def _get_program(reps: int = 1, probe: str | None = None):
    key = (reps, probe)
    if key not in _PROGRAMS:
        _PROGRAMS[key] = _build_program(reps, probe)
    return _PROGRAMS[key]


def _make_masks():
    qi = np.arange(128)[:, None]
    kj = np.arange(WIN)[None, :]
    band = (kj >= qi) & (kj <= qi + BAND - 1)
    mask_s = np.where(band, 0.0, NEG).astype(np.float32)
    mask_0 = np.where(band & (kj >= BAND), 0.0, NEG).astype(np.float32)
    return mask_0, mask_s


def _make_in_maps(states, Wq, Wk):
    x = np.ascontiguousarray(states.reshape(B, S, D2), dtype=np.float32)
    scale = np.float64(D2) ** -0.5
    mT = ((Wq.T.astype(np.float64) @ Wk.astype(np.float64)) * scale).astype(np.float32)
    mask_0, mask_s = _make_masks()

    in_maps = []
    for c in range(8):
        b, h = c // 2, c % 2
        s0 = h * SH
        xpad = np.zeros((S + 2 * BAND, D2), dtype=np.float32)
        xpad[BAND : BAND + S] = x[b]
        xv = np.ascontiguousarray(xpad[s0 : s0 + CTXP])
        xT = np.ascontiguousarray(xv.T)
        in_maps.append(
            {
                "xT": xT,
                "xv": xv,
                "mT": mT,
                "m0": mask_0 if h == 0 else mask_s,
                "ms": mask_s,
            }
        )
    return in_maps


def _assemble(res):
    retrieved = np.empty((B, S, D2), dtype=np.float32)
    w = np.zeros((B, S, S), dtype=np.float32)
    for c in range(8):
        b, h = c // 2, c % 2
        s0 = h * SH
        r = res[c]
        retrieved[b, s0 : s0 + SH] = r["ro"]
        rsums = r["rs"].T.reshape(SH, 1)  # [qi, p] -> row p*128+qi
        wn = r["wb"] / (rsums + 1e-30)
        for p in range(NBLK):
            g0 = s0 + p * 128
            c0 = g0 - BAND
            blk = wn[p * 128 : (p + 1) * 128]
            if c0 < 0:
                w[b, g0 : g0 + 128, 0 : c0 + 192] = blk[:, -c0:]
            else:
                w[b, g0 : g0 + 128, c0 : c0 + 192] = blk
    retrieved[:, 0, :] = 0.0
    w[:, 0, :] = 0.0
    return retrieved.reshape(B, S, D2 // 2, 2), w


def kernel(states: np.ndarray, Wq: np.ndarray, Wk: np.ndarray):
    from concourse.bass_utils import run_bass_kernel_spmd

    in_maps = _make_in_maps(states, Wq, Wk)
    nc = _get_program()
    res = run_bass_kernel_spmd(nc, in_maps, core_ids=list(range(8))).results
    return _assemble(res)


# revision 16
# speedup vs baseline: 81529.4108x; 1.3177x over previous
"""Trainium2 Bass kernel for banded (episodic-memory) attention.

Module computation (B=4, S=4096, D=256, d2=512, band width 64):
    x = states.reshape(B, S, 512)
    q = x @ Wq.T ; k = x @ Wk.T
    scores = q @ k.T / sqrt(512), masked to j in [i-64, i-1]
    w = softmax(scores)  (fully-masked row 0 -> 0)
    retrieved = w @ x
    returns (retrieved.reshape(B,S,256,2), w)

Device strategy (8 cores = 4 batches x 2 sequence halves):
    scores[i,j] = x_i^T (Wq^T Wk) x_j / sqrt(512) = z_i . x_j with
    z_i = (Wq^T Wk)^T x_i -- one fused projection instead of two, and the
    key side of the score matmul is x^T directly.  Per 128-query block the
    live key window is 256 wide (band is 64), so scores are a [128, 256]
    tile with a static additive mask; exp (no max-subtract needed: |s| is
    a few units at most) with fused row-sum; retrieval contracts the 256
    window against the values.  Unnormalized band + row sums go back to
    the host, which normalizes and scatters into the dense [S, S] output
    (all entries outside the band are exactly 0).  All matmuls run in
    float32r (TF32-like, ~1.5e-4 rel err, full PE rate at N>=256).
"""

import sys
from contextlib import ExitStack

if "/opt/trn_rl_repo" not in sys.path:
    sys.path.insert(0, "/opt/trn_rl_repo")

import numpy as np

B, S, D2 = 4, 4096, 512
BAND = 64
SH = S // 2          # 2048 rows per core
CTXP = SH + 2 * BAND  # 2176 padded context rows (= 17 * 128)
NBLK = SH // 128      # 16 query blocks per core
WIN = 256             # key window per query block
NEG = -1e30

_PROGRAMS = {}


def _build_program(reps: int = 1, probe: str | None = None):
    # reps > 1 repeats the whole compute body (same inputs/outputs) inside an
    # on-device For_i loop so a benchmark can difference wall times to
    # isolate per-iteration device time.
    # probe: None (full kernel) | "io" (DMAs only) | "compute" (no DMAs).
    do_io = probe != "compute"
    do_compute = probe != "io"
    import concourse.tile as tile
    from concourse import bacc, mybir
    from concourse.masks import make_identity

    F32 = mybir.dt.float32
    F32R = mybir.dt.float32r
    Exp = mybir.ActivationFunctionType.Exp
    Copy = mybir.ActivationFunctionType.Copy

    nc = bacc.Bacc("TRN2", target_bir_lowering=False, debug=False)

    xT_d = nc.dram_tensor("xT", [D2, CTXP], F32R, kind="ExternalInput").ap()
    xv_d = nc.dram_tensor("xv", [CTXP, D2], F32R, kind="ExternalInput").ap()
    mT_d = nc.dram_tensor("mT", [D2, D2], F32R, kind="ExternalInput").ap()
    m0_d = nc.dram_tensor("m0", [128, WIN], F32, kind="ExternalInput").ap()
    ms_d = nc.dram_tensor("ms", [128, WIN], F32, kind="ExternalInput").ap()
    wb_d = nc.dram_tensor("wb", [SH, 192], F32, kind="ExternalOutput").ap()
    rs_d = nc.dram_tensor("rs", [128, NBLK], F32, kind="ExternalOutput").ap()
    ro_d = nc.dram_tensor("ro", [SH, D2], F32, kind="ExternalOutput").ap()

    with tile.TileContext(nc) as tc:
        with (
            tc.tile_pool(name="const", bufs=1) as cp,
            tc.tile_pool(name="spool", bufs=3) as s_pool,
            tc.tile_pool(name="wpool", bufs=3) as w_pool,
            tc.tile_pool(name="wtpool", bufs=3) as wt_pool,
            tc.tile_pool(name="ropool", bufs=3) as ro_pool,
            tc.tile_pool(name="stats", bufs=6) as st_pool,
            tc.tile_pool(name="pa", bufs=2, space="PSUM") as pa,
            tc.tile_pool(name="ps", bufs=2, space="PSUM") as ps,
            tc.tile_pool(name="pt", bufs=2, space="PSUM") as pt,
            tc.tile_pool(name="pr", bufs=2, space="PSUM") as pr,
            ExitStack() as loop_ctx,
        ):
            ident = cp.tile([128, 128], F32)
            make_identity(nc, ident[:])
            xT_sb = cp.tile([128, 4, CTXP], F32R)
            xv_sb = cp.tile([128, CTXP // 128, D2], F32R)
            mT_sb = cp.tile([128, 4, D2], F32R)
            m0_sb = cp.tile([128, WIN], F32)
            ms_sb = cp.tile([128, WIN], F32)
            zt_sb = cp.tile([128, 4, SH], F32R)
            rs_sb = cp.tile([128, NBLK], F32)

            if not do_io:
                # compute-only probe: one-time zero-init so matmul inputs
                # have writers (outside the bench loop)
                for t in (xT_sb, xv_sb, mT_sb):
                    nc.gpsimd.memset(t[:].bitcast(F32), 0.0)
                for t in (m0_sb, ms_sb):
                    nc.gpsimd.memset(t[:], 0.0)

            if reps > 1:
                loop_ctx.enter_context(tc.For_i(0, reps, 1, staggered_reset=True))

            # ---- input loads (inside the bench loop so per-iteration time
            # includes the real input DMA cost) ----
            if do_io:
                for dc in range(4):
                    nc.sync.dma_start(
                        xT_sb[:, dc, :], xT_d[dc * 128 : (dc + 1) * 128, :]
                    )
                for t in range(CTXP // 128):
                    nc.sync.dma_start(xv_sb[:, t, :], xv_d[t * 128 : (t + 1) * 128, :])
                for dc in range(4):
                    nc.sync.dma_start(
                        mT_sb[:, dc, :], mT_d[dc * 128 : (dc + 1) * 128, :]
                    )
                nc.sync.dma_start(m0_sb[:], m0_d)
                nc.sync.dma_start(ms_sb[:], ms_d)

            if do_compute:
                # ---- phase A: zT = (Wq^T Wk / sqrt(d2))^T-proj of queries ----
                for st in range(4):
                    for ec in range(4):
                        pz = pa.tile([128, 512], F32, tag="pz", name="pz")
                        for dc in range(4):
                            nc.tensor.matmul(
                                pz[:],
                                mT_sb[:, dc, ec * 128 : (ec + 1) * 128],
                                xT_sb[:, dc, BAND + st * 512 : BAND + (st + 1) * 512],
                                start=(dc == 0),
                                stop=(dc == 3),
                            )
                        dst = zt_sb[:, ec, st * 512 : (st + 1) * 512]
                        if (st * 4 + ec) % 2 == 0:
                            nc.scalar.copy(dst, pz[:])
                        else:
                            nc.vector.tensor_copy(dst, pz[:])

            # ---- phase B: per 128-query block ----
            for p in range(NBLK):
                if do_compute:
                    sps = ps.tile([128, WIN], F32, tag="sps", name="sps")
                    for cc in range(4):
                        nc.tensor.matmul(
                            sps[:],
                            zt_sb[:, cc, p * 128 : (p + 1) * 128],
                            xT_sb[:, cc, p * 128 : p * 128 + WIN],
                            start=(cc == 0),
                            stop=(cc == 3),
                        )
                    mask = m0_sb if p == 0 else ms_sb
                    s_sb = s_pool.tile([128, WIN], F32, tag="s", name="s_sb")
                    nc.vector.tensor_add(s_sb[:], sps[:], mask[:])
                    w_sb = w_pool.tile([128, WIN], F32, tag="w", name="w_sb")
                    nc.scalar.activation(
                        w_sb[:], s_sb[:], Exp, accum_out=rs_sb[:, p : p + 1]
                    )
                if do_io:
                    wb_src = w_sb[:, 0:192] if do_compute else xv_sb[:, p, 0:192].bitcast(F32)
                    nc.sync.dma_start(wb_d[p * 128 : (p + 1) * 128, :], wb_src)

                if do_compute:
                    radd = st_pool.tile([128, 1], F32, tag="radd", name="radd")
                    nc.vector.tensor_scalar_add(radd[:], rs_sb[:, p : p + 1], 1e-30)
                    rinv = st_pool.tile([128, 1], F32, tag="rinv", name="rinv")
                    nc.vector.reciprocal(rinv[:], radd[:])

                    tps = pt.tile([128, 2, 128], F32, tag="tps", name="tps")
                    nc.tensor.transpose(tps[:, 0, :], w_sb[:, 0:128], ident[:])
                    nc.tensor.transpose(tps[:, 1, :], w_sb[:, 128:256], ident[:])
                    wt_sb = wt_pool.tile([128, 2, 128], F32R, tag="wt", name="wt_sb")
                    nc.vector.tensor_copy(wt_sb[:], tps[:])

                    rps = pr.tile([128, D2], F32, tag="rps", name="rps")
                    nc.tensor.matmul(
                        rps[:], wt_sb[:, 0, :], xv_sb[:, p, :], start=True, stop=False
                    )
                    nc.tensor.matmul(
                        rps[:],
                        wt_sb[:, 1, :],
                        xv_sb[:, p + 1, :],
                        start=False,
                        stop=True,
                    )
                    ro_sb = ro_pool.tile([128, D2], F32, tag="ro", name="ro_sb")
                    nc.scalar.activation(ro_sb[:], rps[:], Copy, scale=rinv[:])
                if do_io:
                    ro_src = ro_sb[:] if do_compute else xv_sb[:, p, :].bitcast(F32)
                    nc.sync.dma_start(ro_d[p * 128 : (p + 1) * 128, :], ro_src)

            if do_io:
                rs_src = rs_sb[:] if do_compute else xv_sb[:, 0, 0:NBLK].bitcast(F32)
                nc.sync.dma_start(rs_d, rs_src)

    nc.compile()
    return nc


def _get_program(reps: int = 1, probe: str | None = None):
    key = (reps, probe)
    if key not in _PROGRAMS:
        _PROGRAMS[key] = _build_program(reps, probe)
    return _PROGRAMS[key]


def _make_masks():
    qi = np.arange(128)[:, None]
    kj = np.arange(WIN)[None, :]
    band = (kj >= qi) & (kj <= qi + BAND - 1)
    mask_s = np.where(band, 0.0, NEG).astype(np.float32)
    mask_0 = np.where(band & (kj >= BAND), 0.0, NEG).astype(np.float32)
    return mask_0, mask_s


def _make_in_maps(states, Wq, Wk):
    x = np.ascontiguousarray(states.reshape(B, S, D2), dtype=np.float32)
    scale = np.float64(D2) ** -0.5
    mT = ((Wq.T.astype(np.float64) @ Wk.astype(np.float64)) * scale).astype(np.float32)
    mask_0, mask_s = _make_masks()

    in_maps = []
    for c in range(8):
        b, h = c // 2, c % 2
        s0 = h * SH
        xpad = np.zeros((S + 2 * BAND, D2), dtype=np.float32)
        xpad[BAND : BAND + S] = x[b]
        xv = np.ascontiguousarray(xpad[s0 : s0 + CTXP])
        xT = np.ascontiguousarray(xv.T)
        in_maps.append(
            {
                "xT": xT,
                "xv": xv,
                "mT": mT,
                "m0": mask_0 if h == 0 else mask_s,
                "ms": mask_s,
            }
        )
    return in_maps


def _assemble(res):
    retrieved = np.empty((B, S, D2), dtype=np.float32)
    w = np.zeros((B, S, S), dtype=np.float32)
    for c in range(8):
        b, h = c // 2, c % 2
        s0 = h * SH
        r = res[c]
        retrieved[b, s0 : s0 + SH] = r["ro"]
        rsums = r["rs"].T.reshape(SH, 1)  # [qi, p] -> row p*128+qi
        wn = r["wb"] / (rsums + 1e-30)
        for p in range(NBLK):
            g0 = s0 + p * 128
            c0 = g0 - BAND
            blk = wn[p * 128 : (p + 1) * 128]
            if c0 < 0:
                w[b, g0 : g0 + 128, 0 : c0 + 192] = blk[:, -c0:]
            else:
                w[b, g0 : g0 + 128, c0 : c0 + 192] = blk
    retrieved[:, 0, :] = 0.0
    w[:, 0, :] = 0.0
    return retrieved.reshape(B, S, D2 // 2, 2), w


def kernel(states: np.ndarray, Wq: np.ndarray, Wk: np.ndarray):
    from concourse.bass_utils import run_bass_kernel_spmd

    in_maps = _make_in_maps(states, Wq, Wk)
    nc = _get_program()
    res = run_bass_kernel_spmd(nc, in_maps, core_ids=list(range(8))).results
    return _assemble(res)
